# revision 31
# baseline (speedup 1.0000x reference)
"""DeepVONet (2-layer LSTM, H=1000, T=128, B=64, D=1024) on 8 trn2 cores.

Strategy: 8-way model parallel over hidden units (125/core). Layer 2 runs
one timestep behind layer 1 so both layers' new hidden slices ship in ONE
exchange per step. Activations are batch-major [64, *]; layer-1 gemm runs
on PE columns 0-63 and layer-2 gemm on columns 64-127 (tile_position col
tiling) so both stream concurrently. x@W1 is folded into the layer-1 gemm
via host-pre-transposed x, and its matmuls for step t+1 are issued during
step t's exchange wait. Gate columns are reordered to [i,f,o,g] so one
sigmoid covers i,f,o contiguously. Final h2(127) slices are returned
per-core and the tiny Wo projection is done on host.

Exchange ("rdma", default): per-step SBUF->SBUF remote_dma_broadcast of
each core's [h1^T | h2^T] slice to all 8 cores (lib `remote_dma` ucode,
descgen + trigger_dma on gpsimd, consumers gated on the remote rsem).
This replaced the DRAM-bounced gpsimd collective AllGather ("cc"), cutting
the per-step exchange from ~11us to ~6us (3.22ms -> 2.00ms end to end).
Three tricks make it work:
  1. Tile's no-exec schedule sim can't model remote sem arrivals; per-round
     sim-only InstBassCallbacks bump rsem/lsem (+16 each) and are stripped
     from the module before nc.compile().
  2. No then_inc on the snd copy / descgen (HW sync-update slots overflow);
     trigger-after-descgen uses a plain dep resolved via Pool engine ticks,
     and copy-before-descgen is an implicit tile data dep.
  3. A NEFF with no collective gets NO synchronized cross-rank launch from
     the runtime (cores start with multi-ms skew and early cores stall at
     the round-0 rsem wait); a tiny dummy AllGather at program start
     restores the synchronized launch. No sem restore at the end: each
     execution starts with fresh semaphores under this runtime, and a
     negative gpsimd sem_inc crashes it.
"""

import sys
import types

import numpy as np

B = 64
T = 128
D = 1024
H = 1000
NCORES = 8
HS = H // NCORES       # 125 hidden units per core
GS = 4 * HS            # 500 gate columns per core
NXT = D // 128         # 8 x k-tiles
NHB = H // HS          # 8 h blocks (= NCORES)

_CFG = {
    "nsteps": T,        # dev knob: fewer recurrence steps
    "coltile": True,    # PE column-tiling of the two gemms
    "fillers": 0,       # HAM keep-warm mm+copy pairs per step
                        # (2 and 12 both measured slower / failed to build at
                        # full size; the pacing chain cannot bridge the CC
                        # gap without delaying the critical burst)
    "trace": False,     # NTFF profile the run
    "exchange": "rdma",  # "cc" collective AllGather | "rdma" SBUF p2p
                         # (sim-only per-round sem bumps make the schedule
                         # sim converge; callbacks stripped before compile)
}

_CACHE = {}


def _ensure_axon_hooks():
    """bass_utils trace path needs antenv.axon_hooks; shim it if missing."""
    try:
        import antenv.axon_hooks  # noqa: F401
        return
    except ImportError:
        pass
    import trn_agent_boot.trn_boot as tb

    hook = tb._ntff_profile_via_ctypes("/opt/axon/libaxon_pjrt.so")
    m = types.ModuleType("antenv.axon_hooks")
    m.get_axon_ntff_profile_hook = lambda: hook
    sys.modules["antenv.axon_hooks"] = m


def build_program(nsteps, coltile, fillers, with_bias, exchange="cc"):
    import concourse.bacc as bacc
    import concourse.tile as tile
    import concourse.mybir as mybir
    import bass_rust

    def dep(a, b, why="manual"):
        bass_rust.add_dep_helper(a.ins, b.ins, sync=True, reason=why)

    F32 = mybir.dt.float32
    BF16 = mybir.dt.bfloat16
    AF = mybir.ActivationFunctionType
    rdma = exchange == "rdma"

    nc = bacc.Bacc("TRN2", target_bir_lowering=False, debug=False,
                   enable_asserts=False, num_devices=NCORES)
    if rdma:
        rsem = nc.alloc_semaphore("rsem")  # +2 per arriving bcast, +16/round
        lsem = nc.alloc_semaphore("lsem")  # sender-side +16 per drained bcast
        psem = nc.alloc_semaphore("psem")  # +1 per descgen prep
        csem = nc.alloc_semaphore("csem")  # +1 per send-tile copy

        # Tile's no-exec schedule sim cannot model remote sem arrivals (rsem
        # is only incremented by peer cores' DMA payloads; lsem by the local
        # SDMA drain, neither of which the no-exec sim executes). Bump both
        # sems by one round's worth (+16 each) per ship round via sim-only
        # InstBassCallbacks (stripped before nc.compile(), so HW never sees
        # them). Incremental bumps keep sim sem values realistic so the
        # end-of-program restore block stays at the end.
        import concourse.bass_interp as _bi

        def _mk_sim_bump(_r=rsem, _l=lsem):
            def _bump(core_sim):
                from concourse.bass import create_sync_update as _csu
                core_sim.update_semaphore(_csu(_r, 16))
                core_sim.update_semaphore(_csu(_l, 16))
            return _bump

    xT_d = nc.dram_tensor("xT", [T, D, B], BF16, kind="ExternalInput").ap()
    w1s_d = nc.dram_tensor("w1s", [D, GS], BF16, kind="ExternalInput").ap()
    u1s_d = nc.dram_tensor("u1s", [H, GS], BF16, kind="ExternalInput").ap()
    w2u2s_d = nc.dram_tensor("w2u2s", [2 * H, GS], BF16,
                             kind="ExternalInput").ap()
    ident_d = nc.dram_tensor("ident", [128, 128], BF16,
                             kind="ExternalInput").ap()
    if with_bias:
        b1s_d = nc.dram_tensor("b1s", [1, GS], BF16, kind="ExternalInput").ap()
        b2s_d = nc.dram_tensor("b2s", [1, GS], BF16, kind="ExternalInput").ap()
    h2s_d = nc.dram_tensor("h2s", [HS, B], F32, kind="ExternalOutput").ap()
    dump_cc = _CFG.get("dump_cc", False)
    if dump_cc:
        ccdump_d = nc.dram_tensor("ccdump", [2 * H, B], F32,
                                  kind="ExternalOutput").ap()
        zdump_d = nc.dram_tensor("zdump", [128, 1024], F32,
                                 kind="ExternalOutput").ap()

    from contextlib import ExitStack

    with tile.TileContext(nc) as tc, ExitStack() as es:
        cp = es.enter_context(tc.tile_pool(name="const", bufs=1))
        xp = es.enter_context(tc.tile_pool(name="xp", bufs=3))
        gp = es.enter_context(tc.tile_pool(name="gp", bufs=2))
        lp = es.enter_context(tc.tile_pool(name="lstm", bufs=2))
        zp = es.enter_context(tc.tile_pool(name="zp", bufs=2, space="PSUM"))
        tpp = es.enter_context(tc.tile_pool(name="tpp", bufs=2, space="PSUM"))
        kpp = es.enter_context(tc.tile_pool(name="kpp", bufs=1, space="PSUM"))
        dp = es.enter_context(tc.tile_pool(name="dram", bufs=3, space="DRAM"))
        if rdma:
            rp = es.enter_context(tc.tile_pool(name="recv", bufs=4))
            sp = es.enter_context(tc.tile_pool(name="snd", bufs=2))
            POOL = mybir.EngineType.Pool
            pid_sv = nc.partition_id(engines=[POOL])
            from concourse import library_config
            nc.gpsimd.load_library(library_config.remote_dma)
            # Launch-sync barrier: a NEFF containing a collective gets a
            # synchronized cross-rank launch from the runtime (the cc path
            # had this implicitly). Without it, cores launch with multi-ms
            # skew and every early core stalls at the round-0 rsem wait.
            bar_sb = cp.tile([1, 16], F32, tag="bar", name="bar")
            nc.vector.memset(bar_sb[:], 0.0)
            dmy_sb = cp.tile([1, 128], F32, tag="dmy", name="dmy")
            nc.vector.memset(dmy_sb[:], 0.0)
            bar_in = dp.tile([1, 16], F32, tag="barin", name="barin")
            nc.sync.dma_start(out=bar_in[:], in_=bar_sb[:])
            bar_out = dp.tile([NCORES, 16], F32, tag="barout", name="barout")
            nc.gpsimd.collective_compute(
                "AllGather",
                mybir.AluOpType.bypass,
                replica_groups=[list(range(NCORES))],
                ins=[bar_in.opt()],
                outs=[bar_out.opt()],
            )
        if True:
            # ---- resident weights ----
            w1_sb = cp.tile([128, NXT * GS], BF16, tag="w1", name="w1")
            for j in range(NXT):
                nc.sync.dma_start(
                    out=w1_sb[:, GS * j:GS * (j + 1)],
                    in_=w1s_d[128 * j:128 * (j + 1), :])
            u1_sb = cp.tile([128, NHB * GS], BF16, tag="u1", name="u1")
            for j in range(NHB):
                nc.sync.dma_start(
                    out=u1_sb[0:HS, GS * j:GS * (j + 1)],
                    in_=u1s_d[HS * j:HS * (j + 1), :])
            w22_sb = cp.tile([128, 2 * NHB * GS], BF16, tag="w22", name="w22")
            for j in range(2 * NHB):
                nc.sync.dma_start(
                    out=w22_sb[0:HS, GS * j:GS * (j + 1)],
                    in_=w2u2s_d[HS * j:HS * (j + 1), :])
            ident_sb = cp.tile([128, 128], BF16, tag="ident", name="ident")
            nc.sync.dma_start(out=ident_sb[:], in_=ident_d[:])
            if with_bias:
                ones_sb = cp.tile([1, B], BF16, tag="ones", name="ones")
                nc.vector.memset(ones_sb[:], 1.0)
                b1_sb = cp.tile([1, GS], BF16, tag="b1", name="b1")
                nc.sync.dma_start(out=b1_sb[:], in_=b1s_d[:])
                b2_sb = cp.tile([1, GS], BF16, tag="b2", name="b2")
                nc.sync.dma_start(out=b2_sb[:], in_=b2s_d[:])
            if fillers:
                fsrc_sb = cp.tile([128, 512], F32, tag="fsrc", name="fsrc")
                nc.vector.memset(fsrc_sb[:], 0.0)
                fch_sb = cp.tile([128, 128], F32, tag="fch", name="fch",
                                 bufs=2)
                nc.vector.memset(fch_sb[:], 0.0)

            def load_x(tau):
                t_ = xp.tile([128, NXT * B], BF16, tag="xsb", name="xsb")
                nc.sync.dma_start(
                    out=t_[:].rearrange("p (j b) -> p j b", b=B),
                    in_=xT_d[tau].rearrange("(j p) b -> p j b", p=128))
                return t_

            def x_mms(zps, xsb, stop):
                # layer-1 x-part: accumulate into zps[0:64, 0:GS]
                for j in range(NXT):
                    nc.tensor.matmul(
                        zps[0:B, 0:GS],
                        xsb[:, B * j:B * (j + 1)],
                        w1_sb[:, GS * j:GS * (j + 1)],
                        start=(j == 0),
                        stop=(stop and not with_bias and j == NXT - 1),
                        tile_position=(0, 0))
                if with_bias and stop:
                    # tau=0 only; steps >= 1 get their b1 MM in the main loop
                    nc.tensor.matmul(
                        zps[0:B, 0:GS], ones_sb[:], b1_sb[:],
                        start=False, stop=True,
                        tile_position=(0, 0))

            # zps tiles keyed by step; x-part of step tau is issued at tau-1
            zmap = {}
            rvmap = {}
            zmap[0] = zp.tile([128, 512], F32, tag="zps", name="zps")
            xsb0 = load_x(0)
            x_mms(zmap[0], xsb0, stop=True)  # tau=0 has no h-part

            cc_prev = None
            c_prev = None
            out_written = False

            # Iteration i computes z1(i)/h1(i) and z2(i-2)/h2(i-2); the
            # AllGather at i ships [h1(i), h2(i-2)]. The W2 half of z2(i-1)
            # and the x half of z1(i+1) are issued at i, inside the
            # AllGather shadow; only the U1 and U2 matmuls (8+8) sit on the
            # critical path of each step.
            for tau in range(nsteps + 2):
                L1 = tau <= nsteps - 1       # layer-1 step tau exists
                L2 = 2 <= tau <= nsteps + 1  # layer-2 step tau-2 exists
                zps = zmap.pop(tau)

                # gathered h from previous step's exchange
                gsb = None
                wt = None
                if (1 <= tau <= nsteps) or (tau == nsteps + 1 and nsteps >= 2):
                    if rdma:
                        # round tau-1 lands directly in SBUF recv slots;
                        # gate every consumer matmul on its arrival sem. The
                        # per-round sim-only bump callbacks model the remote
                        # increments, so a naked wait is schedulable.
                        gsb = rvmap.pop(tau - 1)
                        wt = nc.tensor.wait_ge(rsem, 16 * tau)
                    else:
                        gsb = gp.tile([128, 16 * B], BF16, tag="gsb",
                                      name="gsb")
                        # two half-DMAs (even h1 blocks / odd h2 blocks): they
                        # land on different HWDGE queues, so the critical U2/A
                        # matmuls each gate on their own small transfer instead
                        # of one shared-queue 2000-descriptor DMA.
                        nj = _CFG.get("reload_split", 4) // 2
                        gv = gsb[0:HS, :].rearrange(
                            "p (jh jl q b) -> p jh jl q b", jh=nj, q=2, b=B)
                        cv = cc_prev[:].rearrange(
                            "(jh jl q p) b -> p jh jl q b", jh=nj, q=2, p=HS)
                        for jh in range(nj):
                            for par in (0, 1):
                                nc.sync.dma_start(out=gv[:, jh, :, par],
                                                  in_=cv[:, jh, :, par])

                # ---- critical-path gemms: U1 (z1(tau)) + U2 (z2(tau-2)) ----
                ajs = list(range(NHB)) if (L1 and tau >= 1) else []
                ujs = list(range(NHB)) if (L2 and tau >= 3) else []
                seq = []
                for i in range(max(len(ujs), len(ajs))):
                    if i < len(ujs):
                        seq.append(("U", ujs[i], i == len(ujs) - 1))
                    if i < len(ajs):
                        seq.append(("A", ajs[i], i == len(ajs) - 1))
                for side, j, last in seq:
                    if side == "A":
                        m = nc.tensor.matmul(
                            zps[0:B, 0:GS],
                            gsb[0:HS, B * (2 * j):B * (2 * j) + B],
                            u1_sb[0:HS, GS * j:GS * (j + 1)],
                            start=False,
                            stop=(last and not with_bias),
                            tile_position=(0, 0))
                    else:
                        m = nc.tensor.matmul(
                            zps[64:128, 0:GS],
                            gsb[0:HS, B * (2 * j + 1):B * (2 * j + 1) + B],
                            w22_sb[0:HS, GS * (2 * j + 1):GS * (2 * j + 2)],
                            start=False,
                            stop=(last and not with_bias),
                            tile_position=(0, 64))
                    if wt is not None:
                        dep(m, wt, "mm after recv arrival")
                if with_bias and (L1 and tau >= 1):
                    nc.tensor.matmul(
                        zps[0:B, 0:GS], ones_sb[:], b1_sb[:],
                        start=False, stop=True,
                        tile_position=(0, 0))
                if with_bias and L2:
                    nc.tensor.matmul(
                        zps[64:128, 0:GS], ones_sb[:], b2_sb[:],
                        start=False, stop=True,
                        tile_position=(0, 64))

                # ---- LSTM pointwise ----
                # layer-1 z in zps[0:64, 0:500]; layer-2 z in
                # zps[64:128, 0:500] (same columns, disjoint partitions) so
                # one 128-row ACTIVATE covers both layers' sigmoid (and one
                # covers both tanh) instead of four per-layer ops.
                z2row = slice(64, 128)
                sig = lp.tile([128, 3 * HS], F32, tag="sig", name="sig")
                tg = lp.tile([128, HS], F32, tag="tg", name="tg")
                if L1 and L2:
                    nc.scalar.activation(sig[0:128, :], zps[0:128, 0:3 * HS],
                                         AF.Sigmoid)
                    nc.scalar.activation(tg[0:128, :], zps[0:128, 3 * HS:GS],
                                         AF.Tanh)
                elif L1:
                    nc.scalar.activation(sig[0:64, :], zps[0:64, 0:3 * HS],
                                         AF.Sigmoid)
                    nc.scalar.activation(tg[0:64, :], zps[0:64, 3 * HS:GS],
                                         AF.Tanh)
                elif L2:
                    nc.scalar.activation(sig[64:128, :],
                                         zps[z2row, 0:3 * HS],
                                         AF.Sigmoid)
                    nc.scalar.activation(tg[64:128, :],
                                         zps[z2row, 3 * HS:GS],
                                         AF.Tanh)

                # rows present this step
                if L1 and L2:
                    rows = slice(0, 128)
                elif L1:
                    rows = slice(0, 64)
                else:
                    rows = slice(64, 128)
                # which rows have a previous cell state?
                pc1 = tau >= 1 and L1
                pc2 = tau >= 3
                if pc1 and pc2:
                    prows = slice(0, 128)
                elif pc1:
                    prows = slice(0, 64)
                elif pc2:
                    prows = slice(64, 128)
                else:
                    prows = None
                # t2 = f*c_prev first: it only needs the sigmoid, so the DVE
                # computes it while ACT is still doing tanh(g)
                if prows is not None:
                    t2 = lp.tile([128, HS], F32, tag="t2", name="t2")
                    nc.vector.tensor_mul(t2[prows, :], sig[prows, HS:2 * HS],
                                         c_prev[prows, :])
                t1 = lp.tile([128, HS], F32, tag="t1", name="t1")
                nc.vector.tensor_mul(t1[rows, :], sig[rows, 0:HS], tg[rows, :])
                c_new = lp.tile([128, HS], F32, tag="c", name="c")
                if prows is not None:
                    nc.vector.tensor_add(c_new[prows, :], t1[prows, :],
                                         t2[prows, :])
                # rows with no previous cell: c = i*g
                nrows = None
                if tau == 0:
                    nrows = slice(0, 64)
                elif tau == 2:
                    nrows = slice(64, 128)
                if nrows is not None:
                    nc.vector.tensor_copy(c_new[nrows, :], t1[nrows, :])
                tc_ = lp.tile([128, HS], F32, tag="tc", name="tc")
                nc.scalar.activation(tc_[rows, :], c_new[rows, :], AF.Tanh)
                hh = lp.tile([128, HS], BF16, tag="hh", name="hh")
                nc.vector.tensor_mul(hh[rows, :], sig[rows, 2 * HS:3 * HS],
                                     tc_[rows, :])
                c_prev = c_new

                # ---- transpose new h slices + ship ----
                ship = (tau <= nsteps - 1) or (tau == nsteps and nsteps >= 2)
                if ship:
                    if tau <= 1:
                        nc.vector.memset(hh[64:128, :], 0.0)  # h2(<0) = 0
                    if tau == nsteps:
                        nc.vector.memset(hh[0:64, :], 0.0)  # no h1(nsteps)
                    tp = tpp.tile([128, 128], BF16, tag="tp", name="tp")
                    tpin = nc.tensor.transpose(tp[0:HS, 0:128], hh[:, :],
                                               ident_sb[:, :])
                    if rdma:
                        snd = sp.tile([128, 128], BF16, tag="snd", name="snd")
                        cw = None
                        if tau >= 2:
                            # snd slot reused from round tau-2: wait drained
                            cw = nc.vector.wait_ge(lsem, 16 * (tau - 1))
                        cpy = nc.vector.tensor_copy(snd[0:HS, :],
                                                    tp[0:HS, 0:128])
                        if cw is not None:
                            dep(cpy, cw, "snd reuse after drain")
                        # NOTE: no csem inc here — the descgen data-deps on
                        # cpy (reads snd), so trigger-after-psem already
                        # implies copy-done; an extra inc overflows cpy's HW
                        # sync-update slots.
                        # sim-only: model this round's remote rsem/lsem
                        # arrivals (stripped before HW compile). no-sync dep:
                        # order-only edge, adds no sem updates to cpy.
                        cb = _bi.add_callback(nc.vector, _mk_sim_bump())
                        bass_rust.add_dep_helper(
                            cb.ins if hasattr(cb, "ins") else cb,
                            cpy.ins, sync=False,
                            reason="sim bump after snd copy")
                        rv = rp.tile([128, NHB * 128], BF16, tag="recv",
                                     name="recv")
                        rvmap[tau] = rv
                        for s in tc.Switch(pid_sv, NCORES):
                            # trigger inside the arm: its prep-ring no_sync
                            # deps then reference only this arm's prep, not
                            # the 7 never-executed sibling arms.
                            pr = nc.gpsimd.remote_dma_broadcast(
                                rv[:, 128 * s:128 * (s + 1)],
                                snd[:, :],
                                rsem,
                                lsem,
                                rdests=[(0, k) for k in range(NCORES)])
                            # no psem: dep(tr, pr) resolves via Pool engine
                            # ticks (tile's trigger_dma dep resolver), adding
                            # no sync-update slots to pr
                            tr = nc.gpsimd.trigger_dma(count=1)
                            dep(tr, pr, "trigger after descgen")
                    else:
                        ccsb = lp.tile([HS, 128], BF16, tag="ccsb",
                                       name="ccsb")
                        nc.vector.tensor_copy(ccsb[:, :], tp[0:HS, 0:128])
                        cc_in = dp.tile([2 * HS, B], BF16, tag="ccin",
                                        name="ccin")
                        # single store DMA: a 2-way split measured ~35us
                        # SLOWER end-to-end (per-DMA fixed cost exceeds the
                        # parallelism gain on a ~700ns transfer)
                        nc.sync.dma_start(
                            out=cc_in[:].rearrange("(g p) b -> p g b", g=2),
                            in_=ccsb[:].rearrange("p (g b) -> p g b", g=2))
                        cc_out = dp.tile([2 * H, B], BF16, tag="ccout",
                                         name="ccout")
                        nc.gpsimd.collective_compute(
                            "AllGather",
                            mybir.AluOpType.bypass,
                            replica_groups=[list(range(NCORES))],
                            ins=[cc_in.opt()],
                            outs=[cc_out.opt()],
                        )
                        cc_prev = cc_out
                if tau == nsteps + 1:
                    # write h2(nsteps-1) slice to output
                    tp = tpp.tile([128, 128], BF16, tag="tp", name="tp")
                    nc.tensor.transpose(tp[0:HS, 0:B], hh[64:128, :],
                                        ident_sb[64:128, 64:128])
                    outsb = lp.tile([HS, B], F32, tag="outsb", name="outsb")
                    nc.vector.tensor_copy(outsb[:, :], tp[0:HS, 0:B])
                    nc.sync.dma_start(out=h2s_d[:], in_=outsb[:])
                    out_written = True
                    if dump_cc:
                        nc.sync.dma_start(out=ccdump_d[:], in_=cc_prev[:])
                        zdsb = lp.tile([128, 1024], F32, tag="zdsb",
                                       name="zdsb")
                        nc.vector.tensor_copy(zdsb[:], zps[:, :])
                        nc.sync.dma_start(out=zdump_d[:], in_=zdsb[:])

                # ---- shadow work during the AllGather wait ----
                if tau + 1 <= nsteps + 1:
                    zmap[tau + 1] = zp.tile([128, 512], F32, tag="zps",
                                            name="zps")
                if tau + 1 <= nsteps - 1:
                    xsb = load_x(tau + 1)
                    x_mms(zmap[tau + 1], xsb, stop=False)
                if 1 <= tau <= nsteps:
                    # W2 half of z2(tau-1) from h1(tau-1) (gsb evens)
                    s = tau - 1
                    for j in range(NHB):
                        m = nc.tensor.matmul(
                            zmap[tau + 1][64:128, 0:GS],
                            gsb[0:HS, B * (2 * j):B * (2 * j) + B],
                            w22_sb[0:HS, GS * (2 * j):GS * (2 * j + 1)],
                            start=(j == 0),
                            stop=(j == NHB - 1 and s == 0 and not with_bias),
                            tile_position=(0, 64))
                        if wt is not None:
                            dep(m, wt, "shadow W2 after recv arrival")

                # ---- HAM keep-warm keepers (rdma) ----
                # The PE idles ~6.7us between the end of this step's dense
                # burst and the next step's U chains (broadcast flight), so
                # the HAM clock gate re-throttles every step and the critical
                # matmuls run at 1.2 GHz (622ns vs 210ns per MM). Pace tiny
                # N=64 keeper MMs across the gap using a chain of small dummy
                # DMAs on the otherwise-idle sync engine (~0.7us apiece,
                # naturally serialized by the engine FIFO), first one gated
                # on the ship transpose so the chain spans the gap.
                if rdma and tau <= nsteps - 1:
                    for f in range(6):
                        dd_dst = dp.tile([1, 128], F32, tag="dmyd",
                                         name="dmyd")
                        ddi = nc.sync.dma_start(out=dd_dst[:], in_=dmy_sb[:])
                        if f == 0:
                            dep(ddi, tpin, "pace chain from ship transpose")
                        kps = kpp.tile([64, 64], F32, tag="kps", name="kps")
                        km = nc.tensor.matmul(kps[0:64, 0:64],
                                              ident_sb[0:64, 0:64],
                                              ident_sb[0:64, 0:64],
                                              start=True, stop=True,
                                              tile_position=(0, 0))
                        dep(km, ddi, "keeper paced by dummy dma")

                # ---- HAM keep-warm fillers ----
                # PE->DVE ping-pong chain paced at ~1.3us/link keeps short
                # matmuls landing on the PE throughout the AllGather wait so
                # the HAM clock gate stays at 8/8.
                if fillers and tau <= nsteps - 1:
                    for f in range(fillers):
                        fps = tpp.tile([128, 512], F32, tag="fps", name="fps",
                                       bufs=1)
                        nc.tensor.matmul(fps[0:128, 0:512],
                                         fch_sb[:, 0:128],
                                         fsrc_sb[:, 0:512],
                                         start=True, stop=True)
                        fch_sb = cp.tile([128, 128], F32, tag="fch",
                                         name="fch", bufs=2)
                        nc.vector.tensor_copy(fch_sb[:], fps[0:128, 0:128])

            assert out_written

            if rdma:
                # No sem restore: under this runtime each NEFF execution
                # starts with fresh semaphores (verified by a twice-run
                # probe), and the negative gpsimd sem_inc crashed the
                # runtime with an opaque INTERNAL error.
                pass

    if rdma:
        # strip the sim-only InstBassCallbacks before HW compile (in place:
        # replacing nc.m wholesale desyncs nc.main_func)
        for fn_ in nc.m.functions:
            for blk_ in fn_.blocks:
                keep = [i_ for i_ in blk_.instructions
                        if not isinstance(i_, (_bi.InstBassTrap,
                                               _bi.InstBassCallback,
                                               _bi.InstBassCallback2))]
                if len(keep) != len(blk_.instructions):
                    blk_.instructions = keep

    nc.compile()
    return nc


def _get_program(with_bias):
    key = (_CFG["nsteps"], _CFG["coltile"], _CFG["fillers"], with_bias,
           _CFG["exchange"])
    if key not in _CACHE:
        _CACHE[key] = build_program(_CFG["nsteps"], _CFG["coltile"],
                                    _CFG["fillers"], with_bias,
                                    exchange=_CFG["exchange"])
    return _CACHE[key]


def kernel(x, W1, U1, b1, W2, U2, b2, Wo, bo):
    from concourse import bass_utils

    x = np.asarray(x, np.float32)
    W1 = np.asarray(W1, np.float32)
    U1 = np.asarray(U1, np.float32)
    b1 = np.asarray(b1, np.float32)
    W2 = np.asarray(W2, np.float32)
    U2 = np.asarray(U2, np.float32)
    b2 = np.asarray(b2, np.float32)
    Wo = np.asarray(Wo, np.float32)
    bo = np.asarray(bo, np.float32)

    with_bias = bool(np.any(b1) or np.any(b2))
    nc = _get_program(with_bias)

    import ml_dtypes
    bf16 = ml_dtypes.bfloat16

    xT = np.ascontiguousarray(x.transpose(1, 2, 0)).astype(bf16)  # [T, D, B]
    ident = np.eye(128, dtype=np.float32).astype(bf16)

    in_maps = []
    for k in range(NCORES):
        idx = np.arange(HS * k, HS * (k + 1))
        cols = np.concatenate([g * H + idx for g in (0, 1, 3, 2)])  # i,f,o,g
        w1s = np.ascontiguousarray(W1[:, cols])
        u1s = np.ascontiguousarray(U1[:, cols])
        w2s = W2[:, cols].reshape(NHB, HS, GS)
        u2s = U2[:, cols].reshape(NHB, HS, GS)
        w2u2 = np.empty((NHB, 2, HS, GS), np.float32)
        w2u2[:, 0] = w2s
        w2u2[:, 1] = u2s
        m = {
            "xT": xT,
            "w1s": w1s.astype(bf16),
            "u1s": u1s.astype(bf16),
            "w2u2s": np.ascontiguousarray(w2u2.reshape(2 * H, GS)).astype(bf16),
            "ident": ident,
        }
        if with_bias:
            m["b1s"] = np.ascontiguousarray(b1[cols][None, :]).astype(bf16)
            m["b2s"] = np.ascontiguousarray(b2[cols][None, :]).astype(bf16)
        in_maps.append(m)

    trace = _CFG["trace"]
    if trace:
        _ensure_axon_hooks()
        bass_utils.upload_artifacts = lambda tmpdir: tmpdir
    res = bass_utils.run_bass_kernel_spmd(
        nc, in_maps, core_ids=list(range(NCORES)), trace=trace)
    kernel.last_exec_time_ns = res.exec_time_ns

    h2T = np.concatenate([res.results[k]["h2s"] for k in range(NCORES)],
                         axis=0)  # [1000, 64]
    out = h2T.T.astype(np.float32) @ Wo + bo
    return out.astype(np.float32)



# revision 32
# speedup vs baseline: 1.1326x; 1.1326x over previous
"""DeepVONet (2-layer LSTM, H=1000, T=128, B=64, D=1024) on 8 trn2 cores.

Strategy: 8-way model parallel over hidden units (125/core). Layer 2 runs
one timestep behind layer 1 so both layers' new hidden slices ship in ONE
exchange per step. Activations are batch-major [64, *]; layer-1 gemm runs
on PE columns 0-63 and layer-2 gemm on columns 64-127 (tile_position col
tiling) so both stream concurrently. x@W1 is folded into the layer-1 gemm
via host-pre-transposed x, and its matmuls for step t+1 are issued during
step t's exchange wait. Gate columns are reordered to [i,f,o,g] so one
sigmoid covers i,f,o contiguously. Final h2(127) slices are returned
per-core and the tiny Wo projection is done on host.

Exchange ("rdma", default): per-step SBUF->SBUF remote_dma_broadcast of
each core's [h1^T | h2^T] slice to all 8 cores (lib `remote_dma` ucode,
descgen + trigger_dma on gpsimd, consumers gated on the remote rsem).
This replaced the DRAM-bounced gpsimd collective AllGather ("cc"), cutting
the per-step exchange from ~11us to ~6us (3.22ms -> 2.00ms end to end).
Three tricks make it work:
  1. Tile's no-exec schedule sim can't model remote sem arrivals; per-round
     sim-only InstBassCallbacks bump rsem/lsem (+16 each) and are stripped
     from the module before nc.compile().
  2. No then_inc on the snd copy / descgen (HW sync-update slots overflow);
     trigger-after-descgen uses a plain dep resolved via Pool engine ticks,
     and copy-before-descgen is an implicit tile data dep.
  3. A NEFF with no collective gets NO synchronized cross-rank launch from
     the runtime (cores start with multi-ms skew and early cores stall at
     the round-0 rsem wait); a tiny dummy AllGather at program start
     restores the synchronized launch. No sem restore at the end: each
     execution starts with fresh semaphores under this runtime, and a
     negative gpsimd sem_inc crashes it.
"""

import sys
import types

import numpy as np

B = 64
T = 128
D = 1024
H = 1000
NCORES = 8
HS = H // NCORES       # 125 hidden units per core
GS = 4 * HS            # 500 gate columns per core
NXT = D // 128         # 8 x k-tiles
NHB = H // HS          # 8 h blocks (= NCORES)

_CFG = {
    "nsteps": T,        # dev knob: fewer recurrence steps
    "coltile": True,    # PE column-tiling of the two gemms
    "fillers": 0,       # HAM keep-warm mm+copy pairs per step
                        # (2 and 12 both measured slower / failed to build at
                        # full size; the pacing chain cannot bridge the CC
                        # gap without delaying the critical burst)
    "trace": False,     # NTFF profile the run
    "exchange": "rdma",  # "cc" collective AllGather | "rdma" SBUF p2p
                         # (sim-only per-round sem bumps make the schedule
                         # sim converge; callbacks stripped before compile)
}

_CACHE = {}


def _ensure_axon_hooks():
    """bass_utils trace path needs antenv.axon_hooks; shim it if missing."""
    try:
        import antenv.axon_hooks  # noqa: F401
        return
    except ImportError:
        pass
    import trn_agent_boot.trn_boot as tb

    hook = tb._ntff_profile_via_ctypes("/opt/axon/libaxon_pjrt.so")
    m = types.ModuleType("antenv.axon_hooks")
    m.get_axon_ntff_profile_hook = lambda: hook
    sys.modules["antenv.axon_hooks"] = m


def build_program(nsteps, coltile, fillers, with_bias, exchange="cc"):
    import concourse.bacc as bacc
    import concourse.tile as tile
    import concourse.mybir as mybir
    import bass_rust

    def dep(a, b, why="manual"):
        bass_rust.add_dep_helper(a.ins, b.ins, sync=True, reason=why)

    F32 = mybir.dt.float32
    BF16 = mybir.dt.bfloat16
    AF = mybir.ActivationFunctionType
    rdma = exchange == "rdma"

    nc = bacc.Bacc("TRN2", target_bir_lowering=False, debug=False,
                   enable_asserts=False, num_devices=NCORES)
    if rdma:
        rsem = nc.alloc_semaphore("rsem")  # +2 per arriving bcast, +16/round
        lsem = nc.alloc_semaphore("lsem")  # sender-side +16 per drained bcast
        psem = nc.alloc_semaphore("psem")  # +1 per descgen prep
        csem = nc.alloc_semaphore("csem")  # +1 per send-tile copy

        # Tile's no-exec schedule sim cannot model remote sem arrivals (rsem
        # is only incremented by peer cores' DMA payloads; lsem by the local
        # SDMA drain, neither of which the no-exec sim executes). Bump both
        # sems by one round's worth (+16 each) per ship round via sim-only
        # InstBassCallbacks (stripped before nc.compile(), so HW never sees
        # them). Incremental bumps keep sim sem values realistic so the
        # end-of-program restore block stays at the end.
        import concourse.bass_interp as _bi

        def _mk_sim_bump(_r=rsem, _l=lsem):
            def _bump(core_sim):
                from concourse.bass import create_sync_update as _csu
                core_sim.update_semaphore(_csu(_r, 16))
                core_sim.update_semaphore(_csu(_l, 16))
            return _bump

    xT_d = nc.dram_tensor("xT", [T, D, B], BF16, kind="ExternalInput").ap()
    w1s_d = nc.dram_tensor("w1s", [D, GS], BF16, kind="ExternalInput").ap()
    u1s_d = nc.dram_tensor("u1s", [H, GS], BF16, kind="ExternalInput").ap()
    w2u2s_d = nc.dram_tensor("w2u2s", [2 * H, GS], BF16,
                             kind="ExternalInput").ap()
    ident_d = nc.dram_tensor("ident", [128, 128], BF16,
                             kind="ExternalInput").ap()
    if with_bias:
        b1s_d = nc.dram_tensor("b1s", [1, GS], BF16, kind="ExternalInput").ap()
        b2s_d = nc.dram_tensor("b2s", [1, GS], BF16, kind="ExternalInput").ap()
    h2s_d = nc.dram_tensor("h2s", [HS, B], F32, kind="ExternalOutput").ap()
    dump_cc = _CFG.get("dump_cc", False)
    if dump_cc:
        ccdump_d = nc.dram_tensor("ccdump", [2 * H, B], F32,
                                  kind="ExternalOutput").ap()
        zdump_d = nc.dram_tensor("zdump", [128, 1024], F32,
                                 kind="ExternalOutput").ap()

    from contextlib import ExitStack

    with tile.TileContext(nc) as tc, ExitStack() as es:
        cp = es.enter_context(tc.tile_pool(name="const", bufs=1))
        xp = es.enter_context(tc.tile_pool(name="xp", bufs=3))
        gp = es.enter_context(tc.tile_pool(name="gp", bufs=2))
        lp = es.enter_context(tc.tile_pool(name="lstm", bufs=2))
        zp = es.enter_context(tc.tile_pool(name="zp", bufs=2, space="PSUM"))
        tpp = es.enter_context(tc.tile_pool(name="tpp", bufs=2, space="PSUM"))
        kpp = es.enter_context(tc.tile_pool(name="kpp", bufs=1, space="PSUM"))
        dp = es.enter_context(tc.tile_pool(name="dram", bufs=3, space="DRAM"))
        if rdma:
            rp = es.enter_context(tc.tile_pool(name="recv", bufs=4))
            sp = es.enter_context(tc.tile_pool(name="snd", bufs=2))
            POOL = mybir.EngineType.Pool
            pid_sv = nc.partition_id(engines=[POOL])
            from concourse import library_config
            nc.gpsimd.load_library(library_config.remote_dma)
            # Launch-sync barrier: a NEFF containing a collective gets a
            # synchronized cross-rank launch from the runtime (the cc path
            # had this implicitly). Without it, cores launch with multi-ms
            # skew and every early core stalls at the round-0 rsem wait.
            bar_sb = cp.tile([1, 16], F32, tag="bar", name="bar")
            nc.vector.memset(bar_sb[:], 0.0)
            dmy_sb = cp.tile([1, 128], F32, tag="dmy", name="dmy")
            nc.vector.memset(dmy_sb[:], 0.0)
            bar_in = dp.tile([1, 16], F32, tag="barin", name="barin")
            nc.sync.dma_start(out=bar_in[:], in_=bar_sb[:])
            bar_out = dp.tile([NCORES, 16], F32, tag="barout", name="barout")
            nc.gpsimd.collective_compute(
                "AllGather",
                mybir.AluOpType.bypass,
                replica_groups=[list(range(NCORES))],
                ins=[bar_in.opt()],
                outs=[bar_out.opt()],
            )
        if True:
            # ---- resident weights ----
            w1_sb = cp.tile([128, NXT * GS], BF16, tag="w1", name="w1")
            for j in range(NXT):
                nc.sync.dma_start(
                    out=w1_sb[:, GS * j:GS * (j + 1)],
                    in_=w1s_d[128 * j:128 * (j + 1), :])
            u1_sb = cp.tile([128, NHB * GS], BF16, tag="u1", name="u1")
            for j in range(NHB):
                nc.sync.dma_start(
                    out=u1_sb[0:HS, GS * j:GS * (j + 1)],
                    in_=u1s_d[HS * j:HS * (j + 1), :])
            w22_sb = cp.tile([128, 2 * NHB * GS], BF16, tag="w22", name="w22")
            for j in range(2 * NHB):
                nc.sync.dma_start(
                    out=w22_sb[0:HS, GS * j:GS * (j + 1)],
                    in_=w2u2s_d[HS * j:HS * (j + 1), :])
            ident_sb = cp.tile([128, 128], BF16, tag="ident", name="ident")
            nc.sync.dma_start(out=ident_sb[:], in_=ident_d[:])
            if with_bias:
                ones_sb = cp.tile([1, B], BF16, tag="ones", name="ones")
                nc.vector.memset(ones_sb[:], 1.0)
                b1_sb = cp.tile([1, GS], BF16, tag="b1", name="b1")
                nc.sync.dma_start(out=b1_sb[:], in_=b1s_d[:])
                b2_sb = cp.tile([1, GS], BF16, tag="b2", name="b2")
                nc.sync.dma_start(out=b2_sb[:], in_=b2s_d[:])
            if fillers:
                fsrc_sb = cp.tile([128, 512], F32, tag="fsrc", name="fsrc")
                nc.vector.memset(fsrc_sb[:], 0.0)
                fch_sb = cp.tile([128, 128], F32, tag="fch", name="fch",
                                 bufs=2)
                nc.vector.memset(fch_sb[:], 0.0)

            def load_x(tau):
                t_ = xp.tile([128, NXT * B], BF16, tag="xsb", name="xsb")
                nc.sync.dma_start(
                    out=t_[:].rearrange("p (j b) -> p j b", b=B),
                    in_=xT_d[tau].rearrange("(j p) b -> p j b", p=128))
                return t_

            def x_mms(zps, xsb, stop):
                # layer-1 x-part: accumulate into zps[0:64, 0:GS]
                for j in range(NXT):
                    nc.tensor.matmul(
                        zps[0:B, 0:GS],
                        xsb[:, B * j:B * (j + 1)],
                        w1_sb[:, GS * j:GS * (j + 1)],
                        start=(j == 0),
                        stop=(stop and not with_bias and j == NXT - 1),
                        tile_position=(0, 0))
                if with_bias and stop:
                    # tau=0 only; steps >= 1 get their b1 MM in the main loop
                    nc.tensor.matmul(
                        zps[0:B, 0:GS], ones_sb[:], b1_sb[:],
                        start=False, stop=True,
                        tile_position=(0, 0))

            # zps tiles keyed by step; x-part of step tau is issued at tau-1
            zmap = {}
            rvmap = {}
            zmap[0] = zp.tile([128, 512], F32, tag="zps", name="zps")
            xsb0 = load_x(0)
            x_mms(zmap[0], xsb0, stop=True)  # tau=0 has no h-part

            cc_prev = None
            c_prev = None
            out_written = False

            # Iteration i computes z1(i)/h1(i) and z2(i-2)/h2(i-2); the
            # AllGather at i ships [h1(i), h2(i-2)]. The W2 half of z2(i-1)
            # and the x half of z1(i+1) are issued at i, inside the
            # AllGather shadow; only the U1 and U2 matmuls (8+8) sit on the
            # critical path of each step.
            for tau in range(nsteps + 2):
                L1 = tau <= nsteps - 1       # layer-1 step tau exists
                L2 = 2 <= tau <= nsteps + 1  # layer-2 step tau-2 exists
                zps = zmap.pop(tau)

                # gathered h from previous step's exchange
                gsb = None
                wt = None
                if (1 <= tau <= nsteps) or (tau == nsteps + 1 and nsteps >= 2):
                    if rdma:
                        # round tau-1 lands directly in SBUF recv slots;
                        # gate every consumer matmul on its arrival sem. The
                        # per-round sim-only bump callbacks model the remote
                        # increments, so a naked wait is schedulable.
                        gsb = rvmap.pop(tau - 1)
                        wt = nc.tensor.wait_ge(rsem, 16 * tau)
                    else:
                        gsb = gp.tile([128, 16 * B], BF16, tag="gsb",
                                      name="gsb")
                        # two half-DMAs (even h1 blocks / odd h2 blocks): they
                        # land on different HWDGE queues, so the critical U2/A
                        # matmuls each gate on their own small transfer instead
                        # of one shared-queue 2000-descriptor DMA.
                        nj = _CFG.get("reload_split", 4) // 2
                        gv = gsb[0:HS, :].rearrange(
                            "p (jh jl q b) -> p jh jl q b", jh=nj, q=2, b=B)
                        cv = cc_prev[:].rearrange(
                            "(jh jl q p) b -> p jh jl q b", jh=nj, q=2, p=HS)
                        for jh in range(nj):
                            for par in (0, 1):
                                nc.sync.dma_start(out=gv[:, jh, :, par],
                                                  in_=cv[:, jh, :, par])

                # ---- critical-path gemms: U1 (z1(tau)) + U2 (z2(tau-2)) ----
                ajs = list(range(NHB)) if (L1 and tau >= 1) else []
                ujs = list(range(NHB)) if (L2 and tau >= 3) else []
                seq = []
                for i in range(max(len(ujs), len(ajs))):
                    if i < len(ujs):
                        seq.append(("U", ujs[i], i == len(ujs) - 1))
                    if i < len(ajs):
                        seq.append(("A", ajs[i], i == len(ajs) - 1))
                for side, j, last in seq:
                    if side == "A":
                        m = nc.tensor.matmul(
                            zps[0:B, 0:GS],
                            gsb[0:HS, B * (2 * j):B * (2 * j) + B],
                            u1_sb[0:HS, GS * j:GS * (j + 1)],
                            start=False,
                            stop=(last and not with_bias),
                            tile_position=(0, 0))
                    else:
                        m = nc.tensor.matmul(
                            zps[64:128, 0:GS],
                            gsb[0:HS, B * (2 * j + 1):B * (2 * j + 1) + B],
                            w22_sb[0:HS, GS * (2 * j + 1):GS * (2 * j + 2)],
                            start=False,
                            stop=(last and not with_bias),
                            tile_position=(0, 64))
                    if wt is not None:
                        dep(m, wt, "mm after recv arrival")
                if with_bias and (L1 and tau >= 1):
                    nc.tensor.matmul(
                        zps[0:B, 0:GS], ones_sb[:], b1_sb[:],
                        start=False, stop=True,
                        tile_position=(0, 0))
                if with_bias and L2:
                    nc.tensor.matmul(
                        zps[64:128, 0:GS], ones_sb[:], b2_sb[:],
                        start=False, stop=True,
                        tile_position=(0, 64))

                # ---- LSTM pointwise ----
                # layer-1 z in zps[0:64, 0:500]; layer-2 z in
                # zps[64:128, 0:500] (same columns, disjoint partitions) so
                # one 128-row ACTIVATE covers both layers' sigmoid (and one
                # covers both tanh) instead of four per-layer ops.
                z2row = slice(64, 128)
                sig = lp.tile([128, 3 * HS], F32, tag="sig", name="sig")
                tg = lp.tile([128, HS], F32, tag="tg", name="tg")
                if L1 and L2:
                    nc.scalar.activation(sig[0:128, :], zps[0:128, 0:3 * HS],
                                         AF.Sigmoid)
                    nc.scalar.activation(tg[0:128, :], zps[0:128, 3 * HS:GS],
                                         AF.Tanh)
                elif L1:
                    nc.scalar.activation(sig[0:64, :], zps[0:64, 0:3 * HS],
                                         AF.Sigmoid)
                    nc.scalar.activation(tg[0:64, :], zps[0:64, 3 * HS:GS],
                                         AF.Tanh)
                elif L2:
                    nc.scalar.activation(sig[64:128, :],
                                         zps[z2row, 0:3 * HS],
                                         AF.Sigmoid)
                    nc.scalar.activation(tg[64:128, :],
                                         zps[z2row, 3 * HS:GS],
                                         AF.Tanh)

                # rows present this step
                if L1 and L2:
                    rows = slice(0, 128)
                elif L1:
                    rows = slice(0, 64)
                else:
                    rows = slice(64, 128)
                # which rows have a previous cell state?
                pc1 = tau >= 1 and L1
                pc2 = tau >= 3
                if pc1 and pc2:
                    prows = slice(0, 128)
                elif pc1:
                    prows = slice(0, 64)
                elif pc2:
                    prows = slice(64, 128)
                else:
                    prows = None
                # t2 = f*c_prev first: it only needs the sigmoid, so the DVE
                # computes it while ACT is still doing tanh(g)
                if prows is not None:
                    t2 = lp.tile([128, HS], F32, tag="t2", name="t2")
                    nc.vector.tensor_mul(t2[prows, :], sig[prows, HS:2 * HS],
                                         c_prev[prows, :])
                t1 = lp.tile([128, HS], F32, tag="t1", name="t1")
                nc.vector.tensor_mul(t1[rows, :], sig[rows, 0:HS], tg[rows, :])
                c_new = lp.tile([128, HS], F32, tag="c", name="c")
                if prows is not None:
                    nc.vector.tensor_add(c_new[prows, :], t1[prows, :],
                                         t2[prows, :])
                # rows with no previous cell: c = i*g
                nrows = None
                if tau == 0:
                    nrows = slice(0, 64)
                elif tau == 2:
                    nrows = slice(64, 128)
                if nrows is not None:
                    nc.vector.tensor_copy(c_new[nrows, :], t1[nrows, :])
                tc_ = lp.tile([128, HS], F32, tag="tc", name="tc")
                nc.scalar.activation(tc_[rows, :], c_new[rows, :], AF.Tanh)
                hh = lp.tile([128, HS], BF16, tag="hh", name="hh")
                nc.vector.tensor_mul(hh[rows, :], sig[rows, 2 * HS:3 * HS],
                                     tc_[rows, :])
                c_prev = c_new

                # ---- transpose new h slices + ship ----
                ship = (tau <= nsteps - 1) or (tau == nsteps and nsteps >= 2)
                if ship:
                    if tau <= 1:
                        nc.vector.memset(hh[64:128, :], 0.0)  # h2(<0) = 0
                    if tau == nsteps:
                        nc.vector.memset(hh[0:64, :], 0.0)  # no h1(nsteps)
                    tp = tpp.tile([128, 128], BF16, tag="tp", name="tp")
                    tpin = nc.tensor.transpose(tp[0:HS, 0:128], hh[:, :],
                                               ident_sb[:, :])
                    if rdma:
                        snd = sp.tile([128, 128], BF16, tag="snd", name="snd")
                        cw = None
                        if tau >= 2:
                            # snd slot reused from round tau-2: wait drained
                            cw = nc.vector.wait_ge(lsem, 16 * (tau - 1))
                        cpy = nc.vector.tensor_copy(snd[0:HS, :],
                                                    tp[0:HS, 0:128])
                        if cw is not None:
                            dep(cpy, cw, "snd reuse after drain")
                        # NOTE: no csem inc here — the descgen data-deps on
                        # cpy (reads snd), so trigger-after-psem already
                        # implies copy-done; an extra inc overflows cpy's HW
                        # sync-update slots.
                        # sim-only: model this round's remote rsem/lsem
                        # arrivals (stripped before HW compile). no-sync dep:
                        # order-only edge, adds no sem updates to cpy.
                        cb = _bi.add_callback(nc.vector, _mk_sim_bump())
                        bass_rust.add_dep_helper(
                            cb.ins if hasattr(cb, "ins") else cb,
                            cpy.ins, sync=False,
                            reason="sim bump after snd copy")
                        rv = rp.tile([128, NHB * 128], BF16, tag="recv",
                                     name="recv")
                        rvmap[tau] = rv
                        for s in tc.Switch(pid_sv, NCORES):
                            # trigger inside the arm: its prep-ring no_sync
                            # deps then reference only this arm's prep, not
                            # the 7 never-executed sibling arms.
                            pr = nc.gpsimd.remote_dma_broadcast(
                                rv[:, 128 * s:128 * (s + 1)],
                                snd[:, :],
                                rsem,
                                lsem,
                                rdests=[(0, k) for k in range(NCORES)])
                            # no psem: dep(tr, pr) resolves via Pool engine
                            # ticks (tile's trigger_dma dep resolver), adding
                            # no sync-update slots to pr
                            tr = nc.gpsimd.trigger_dma(count=1)
                            dep(tr, pr, "trigger after descgen")
                    else:
                        ccsb = lp.tile([HS, 128], BF16, tag="ccsb",
                                       name="ccsb")
                        nc.vector.tensor_copy(ccsb[:, :], tp[0:HS, 0:128])
                        cc_in = dp.tile([2 * HS, B], BF16, tag="ccin",
                                        name="ccin")
                        # single store DMA: a 2-way split measured ~35us
                        # SLOWER end-to-end (per-DMA fixed cost exceeds the
                        # parallelism gain on a ~700ns transfer)
                        nc.sync.dma_start(
                            out=cc_in[:].rearrange("(g p) b -> p g b", g=2),
                            in_=ccsb[:].rearrange("p (g b) -> p g b", g=2))
                        cc_out = dp.tile([2 * H, B], BF16, tag="ccout",
                                         name="ccout")
                        nc.gpsimd.collective_compute(
                            "AllGather",
                            mybir.AluOpType.bypass,
                            replica_groups=[list(range(NCORES))],
                            ins=[cc_in.opt()],
                            outs=[cc_out.opt()],
                        )
                        cc_prev = cc_out
                if tau == nsteps + 1:
                    # write h2(nsteps-1) slice to output
                    tp = tpp.tile([128, 128], BF16, tag="tp", name="tp")
                    nc.tensor.transpose(tp[0:HS, 0:B], hh[64:128, :],
                                        ident_sb[64:128, 64:128])
                    outsb = lp.tile([HS, B], F32, tag="outsb", name="outsb")
                    nc.vector.tensor_copy(outsb[:, :], tp[0:HS, 0:B])
                    nc.sync.dma_start(out=h2s_d[:], in_=outsb[:])
                    out_written = True
                    if dump_cc:
                        nc.sync.dma_start(out=ccdump_d[:], in_=cc_prev[:])
                        zdsb = lp.tile([128, 1024], F32, tag="zdsb",
                                       name="zdsb")
                        nc.vector.tensor_copy(zdsb[:], zps[:, :])
                        nc.sync.dma_start(out=zdump_d[:], in_=zdsb[:])

                # ---- shadow work during the AllGather wait ----
                if tau + 1 <= nsteps + 1:
                    zmap[tau + 1] = zp.tile([128, 512], F32, tag="zps",
                                            name="zps")
                if tau + 1 <= nsteps - 1:
                    xsb = load_x(tau + 1)
                    x_mms(zmap[tau + 1], xsb, stop=False)
                if 1 <= tau <= nsteps:
                    # W2 half of z2(tau-1) from h1(tau-1) (gsb evens)
                    s = tau - 1
                    for j in range(NHB):
                        m = nc.tensor.matmul(
                            zmap[tau + 1][64:128, 0:GS],
                            gsb[0:HS, B * (2 * j):B * (2 * j) + B],
                            w22_sb[0:HS, GS * (2 * j):GS * (2 * j + 1)],
                            start=(j == 0),
                            stop=(j == NHB - 1 and s == 0 and not with_bias),
                            tile_position=(0, 64))
                        if wt is not None:
                            dep(m, wt, "shadow W2 after recv arrival")

                # ---- HAM keep-warm keepers (rdma) ----
                # The PE idles ~6.7us between the end of this step's dense
                # burst and the next step's U chains (broadcast flight), so
                # the HAM clock gate re-throttles every step and the critical
                # matmuls run at 1.2 GHz (622ns vs 210ns per MM). Pace tiny
                # N=64 keeper MMs across the gap using a chain of small dummy
                # DMAs on the otherwise-idle sync engine (~0.7us apiece,
                # naturally serialized by the engine FIFO), first one gated
                # on the ship transpose so the chain spans the gap.
                # (measured: 6 keepers/step at 2.11ms vs 2.00ms without —
                # the pacing DMAs/keeper MMs cost more than the warm-clock
                # gain here, so the chain is disabled)
                if False and rdma and tau <= nsteps - 1:
                    for f in range(6):
                        dd_dst = dp.tile([1, 128], F32, tag="dmyd",
                                         name="dmyd")
                        ddi = nc.sync.dma_start(out=dd_dst[:], in_=dmy_sb[:])
                        if f == 0:
                            dep(ddi, tpin, "pace chain from ship transpose")
                        kps = kpp.tile([64, 64], F32, tag="kps", name="kps")
                        km = nc.tensor.matmul(kps[0:64, 0:64],
                                              ident_sb[0:64, 0:64],
                                              ident_sb[0:64, 0:64],
                                              start=True, stop=True,
                                              tile_position=(0, 0))
                        dep(km, ddi, "keeper paced by dummy dma")

                # ---- HAM keep-warm fillers ----
                # PE->DVE ping-pong chain paced at ~1.3us/link keeps short
                # matmuls landing on the PE throughout the AllGather wait so
                # the HAM clock gate stays at 8/8.
                if fillers and tau <= nsteps - 1:
                    for f in range(fillers):
                        fps = tpp.tile([128, 512], F32, tag="fps", name="fps",
                                       bufs=1)
                        nc.tensor.matmul(fps[0:128, 0:512],
                                         fch_sb[:, 0:128],
                                         fsrc_sb[:, 0:512],
                                         start=True, stop=True)
                        fch_sb = cp.tile([128, 128], F32, tag="fch",
                                         name="fch", bufs=2)
                        nc.vector.tensor_copy(fch_sb[:], fps[0:128, 0:128])

            assert out_written

            if rdma:
                # No sem restore: under this runtime each NEFF execution
                # starts with fresh semaphores (verified by a twice-run
                # probe), and the negative gpsimd sem_inc crashed the
                # runtime with an opaque INTERNAL error.
                pass

    if rdma:
        # strip the sim-only InstBassCallbacks before HW compile (in place:
        # replacing nc.m wholesale desyncs nc.main_func)
        for fn_ in nc.m.functions:
            for blk_ in fn_.blocks:
                keep = [i_ for i_ in blk_.instructions
                        if not isinstance(i_, (_bi.InstBassTrap,
                                               _bi.InstBassCallback,
                                               _bi.InstBassCallback2))]
                if len(keep) != len(blk_.instructions):
                    blk_.instructions = keep

    nc.compile()
    return nc


def _get_program(with_bias):
    key = (_CFG["nsteps"], _CFG["coltile"], _CFG["fillers"], with_bias,
           _CFG["exchange"])
    if key not in _CACHE:
        _CACHE[key] = build_program(_CFG["nsteps"], _CFG["coltile"],
                                    _CFG["fillers"], with_bias,
                                    exchange=_CFG["exchange"])
    return _CACHE[key]


def kernel(x, W1, U1, b1, W2, U2, b2, Wo, bo):
    from concourse import bass_utils

    x = np.asarray(x, np.float32)
    W1 = np.asarray(W1, np.float32)
    U1 = np.asarray(U1, np.float32)
    b1 = np.asarray(b1, np.float32)
    W2 = np.asarray(W2, np.float32)
    U2 = np.asarray(U2, np.float32)
    b2 = np.asarray(b2, np.float32)
    Wo = np.asarray(Wo, np.float32)
    bo = np.asarray(bo, np.float32)

    with_bias = bool(np.any(b1) or np.any(b2))
    nc = _get_program(with_bias)

    import ml_dtypes
    bf16 = ml_dtypes.bfloat16

    xT = np.ascontiguousarray(x.transpose(1, 2, 0)).astype(bf16)  # [T, D, B]
    ident = np.eye(128, dtype=np.float32).astype(bf16)

    in_maps = []
    for k in range(NCORES):
        idx = np.arange(HS * k, HS * (k + 1))
        cols = np.concatenate([g * H + idx for g in (0, 1, 3, 2)])  # i,f,o,g
        w1s = np.ascontiguousarray(W1[:, cols])
        u1s = np.ascontiguousarray(U1[:, cols])
        w2s = W2[:, cols].reshape(NHB, HS, GS)
        u2s = U2[:, cols].reshape(NHB, HS, GS)
        w2u2 = np.empty((NHB, 2, HS, GS), np.float32)
        w2u2[:, 0] = w2s
        w2u2[:, 1] = u2s
        m = {
            "xT": xT,
            "w1s": w1s.astype(bf16),
            "u1s": u1s.astype(bf16),
            "w2u2s": np.ascontiguousarray(w2u2.reshape(2 * H, GS)).astype(bf16),
            "ident": ident,
        }
        if with_bias:
            m["b1s"] = np.ascontiguousarray(b1[cols][None, :]).astype(bf16)
            m["b2s"] = np.ascontiguousarray(b2[cols][None, :]).astype(bf16)
        in_maps.append(m)

    trace = _CFG["trace"]
    if trace:
        _ensure_axon_hooks()
        bass_utils.upload_artifacts = lambda tmpdir: tmpdir
    res = bass_utils.run_bass_kernel_spmd(
        nc, in_maps, core_ids=list(range(NCORES)), trace=trace)
    kernel.last_exec_time_ns = res.exec_time_ns

    h2T = np.concatenate([res.results[k]["h2s"] for k in range(NCORES)],
                         axis=0)  # [1000, 64]
    out = h2T.T.astype(np.float32) @ Wo + bo
    return out.astype(np.float32)

